# revision 20
# baseline (speedup 1.0000x reference)
# MoE layer (all-experts dense MLP + weighted combine) on 8 TRN2 NeuronCores.
#
# Reference, for every token b (B=65536 total):
#   h_e   = relu(x @ W1[e] + b1[e])          e = 0..7
#   y_e   = h_e @ W2[e] + b2[e]
#   out_b = sum_e weights[b, e] * y_e
#
# Strategy (data-parallel over B, expert params replicated):
#   - Shard B across the 8 cores (8192 tokens each).
#   - Hidden dim stays on partitions ("hdim-major"):
#       L1:  z_e^T[h, b]  = W1_e^T @ x^T          (W1 chunks stationary)
#       h_e^T             = relu(z_e^T + b1_e)    (ACT/DVE, per-partition bias)
#       hs_e^T            = h_e^T * w_bcast_e     (DVE)
#       out^T[o, b]       = sum_{e,k} W2_chunk^T @ hs_chunk + b2^T @ w^T
#     accumulated in a single PSUM bank - the expert combine is free.
#   - w_bcast_e comes from a DMA with a step-0 partition AP reading the
#     host-transposed weights row straight from DRAM.
#
# v2 scheduling (driven by the neuron-profile trace of v1):
#   - Const weights are DMA'd in per-expert chunks spread over the Pool /
#     DVE / SP queues in first-use order, so the first L1 matmul fires at
#     ~2us instead of ~23us (v1 serialized 1MB of consts on one queue).
#   - The PE stream is software-pipelined ACROSS tiles: L1(t,e) leads
#     L2(t,e-1), and tile t+1's first L1 runs before tile t's last L2, so
#     the relu->mul chain of the last expert no longer stalls the PE at
#     every tile boundary (~1.3us/tile in v1).
#   - The PSUM out-accumulator drain is split in halves on ACT and DVE so
#     the single po buffer recycles ~0.6us faster.
#   - x^T tiles are split into 512-token halves so the first matmul only
#     waits for 128KB, and the out DMA starts after the first half-copy.
import numpy as np
import ml_dtypes

import concourse.bass as bass
import concourse.mybir as mybir
import concourse.tile as tile
import concourse.bass_utils as _bu
from concourse.bass_utils import run_bass_kernel_spmd


E, D_IN, D_HID, D_OUT, B = 8, 128, 256, 128, 65536
N_CORES = 8
B_SHARD = B // N_CORES  # 8192
NB = 1024               # tokens per tile
NCHUNK = D_HID // 128   # 2 hidden-dim chunks per expert

BF16 = mybir.dt.bfloat16
F32 = mybir.dt.float32
RELU = mybir.ActivationFunctionType.Relu

# engine split knobs (tuned against the profile)
# relu chunks (c = 2*e + m) assigned fully to DVE; the rest go to ACT.
RELU_DVE = {5, 9, 13, 15}
# relu chunks split in 512-token halves: j0 on ACT, j1 on DVE.
RELU_JSPLIT = set()
# experts whose h*w multiply runs on GpSimd as four [128,512] pieces
# (the Pool engine is otherwise idle; these experts' L2 slots give ~4us
# of slack, enough for Pool's ~1us/piece latency).
MUL_POOL = {5, 6}

_nc_cache = {}


def dedup_ldw(nc):
    """Drop redundant PE weight loads.

    Tile emits an InstLdweights before every InstMatmult; consecutive
    matmuls over the two 512-token halves of a tile reuse the same
    stationary weights, so the second load is a hardware no-op (weights
    persist in the PE array until the next load). Deleting it saves PE
    queue time; its semaphore waits/updates are carried onto the next PE
    instruction (legalize_waits splits any overflow afterwards).
    """
    for f in nc.m.functions:
        for b in f.blocks:
            il = b.instructions
            out = []
            last_key = None
            carry_w, carry_u = [], []
            for inst in il:
                if inst.engine != mybir.EngineType.PE:
                    out.append(inst)
                    continue
                if isinstance(inst, mybir.InstLdweights):
                    key = str(inst.ins[0])
                    if key == last_key:
                        si = inst.sync_info
                        if si is not None:
                            carry_w.extend(list(si.on_wait))
                            carry_u.extend(list(si.on_update))
                        continue
                    last_key = key
                elif not isinstance(
                    inst, (mybir.InstMatmult, mybir.InstEventSemaphore)
                ):
                    last_key = None
                if carry_w or carry_u:
                    si = inst.sync_info
                    w = (list(si.on_wait) if si else []) + carry_w
                    u = (list(si.on_update) if si else []) + carry_u
                    inst.sync_info = mybir.SyncInfo(on_wait=w, on_update=u)
                    carry_w, carry_u = [], []
                out.append(inst)
            il[:] = out
    return nc


def legalize_waits(nc):
    """Split multi-wait instructions into standalone EventSemaphore waits.

    The walrus build in this container enforces the hardware sync-slot
    budget strictly: a normal instruction holds at most 1 sem wait (+1
    update); an EventSemaphore instruction holds 2. Tile's scheduler
    attaches up to 3 waits per instruction (and ~11 on the kernel-tail
    drain), which codegen rejects with "Too many sync wait commands".
    Hoisting the excess waits into standalone EventSemaphore instructions
    immediately before the op (same engine queue, so they gate execution
    identically) makes the program legal without changing semantics.
    """
    for f in nc.m.functions:
        for b in f.blocks:
            il = b.instructions
            out = []
            changed = False
            for inst in il:
                si = inst.sync_info
                if si is not None:
                    waits = list(si.on_wait)
                    upds = list(si.on_update)
                    assert len(upds) <= 1, f"{inst.name}: {len(upds)} updates"
                    cap = 2 if isinstance(inst, mybir.InstEventSemaphore) else 1
                    if len(waits) > cap:
                        extra, keep = waits[:-cap], waits[-cap:]
                        k = 0
                        while extra:
                            chunk, extra = extra[:2], extra[2:]
                            ev = mybir.InstEventSemaphore(
                                name=f"{inst.name}-lw{k}", ins=[], outs=[]
                            )
                            ev.engine = inst.engine
                            ev.sync_info = mybir.SyncInfo(
                                on_wait=chunk, on_update=[]
                            )
                            out.append(ev)
                            k += 1
                        inst.sync_info = mybir.SyncInfo(
                            on_wait=keep, on_update=upds
                        )
                        changed = True
                out.append(inst)
            if changed:
                il[:] = out
    return nc


def _rep2(ap_2d, n):
    """View a [128, F] AP as [128, n, F] with a step-0 middle dim."""
    return bass.AP(
        tensor=ap_2d.tensor,
        offset=ap_2d.offset,
        ap=[ap_2d.ap[0], [0, n], ap_2d.ap[1]],
    )


def build_nc(b_shard=B_SHARD, nb=NB, legalize=True):
    assert b_shard % nb == 0
    n_tiles = b_shard // nb
    nsub = nb // 512
    assert nsub == 2, "schedule assumes 1024-token tiles"
    nc = bass.Bass(trn_type="TRN2")

    xt = nc.dram_tensor("xt", [D_IN, b_shard], BF16, kind="ExternalInput").ap()
    wt = nc.dram_tensor("wt", [E, b_shard], BF16, kind="ExternalInput").ap()
    # W1 laid out [i, (e, m), h']: chunk (e, m) is lhsT for z_e rows m*128..
    w1l = nc.dram_tensor("w1l", [D_IN, E * NCHUNK, 128], BF16, kind="ExternalInput").ap()
    # b1 laid out [p, (e, m)] = b1[e, m*128 + p]
    b1l = nc.dram_tensor("b1l", [128, E * NCHUNK], F32, kind="ExternalInput").ap()
    # W2 laid out [h', (e, k), o]: chunk (e, k) is lhsT contracting h rows k*128..
    w2l = nc.dram_tensor("w2l", [128, E * NCHUNK, D_OUT], BF16, kind="ExternalInput").ap()
    b2 = nc.dram_tensor("b2", [E, D_OUT], BF16, kind="ExternalInput").ap()
    # out is produced in bf16 (the host upcasts); this halves the output DMA
    # bytes on a DMA fabric that is within ~10% of saturation, and costs
    # ~0.2% relative error against a 2% gate.
    outT = nc.dram_tensor("outT", [D_OUT, b_shard], BF16, kind="ExternalOutput").ap()

    with tile.TileContext(nc) as tc:
        with (
            tc.tile_pool(name="consts", bufs=1) as consts,
            tc.tile_pool(name="xt_p", bufs=6) as xt_p,
            tc.tile_pool(name="wt_p", bufs=3) as wt_p,
            tc.tile_pool(name="wbc_p", bufs=6) as wbc_p,
            tc.tile_pool(name="h_p", bufs=6) as h_p,
            tc.tile_pool(name="hs_p", bufs=6) as hs_p,
            tc.tile_pool(name="ot_p", bufs=6) as ot_p,
            tc.tile_pool(name="z_ps", bufs=3, space="PSUM") as z_ps,
            tc.tile_pool(name="o_ps", bufs=1, space="PSUM") as o_ps,
        ):
            # ---- consts, chunked + spread over queues in first-use order ----
            # Pool queue: b1, b2, W1 for e0-e3, then the wbc stream (below).
            b1_dma = consts.tile([128, E * NCHUNK], F32, tag="b1_dma")
            nc.gpsimd.dma_start(b1_dma, b1l)
            b2_sb = consts.tile([E, D_OUT], BF16, tag="b2")
            nc.gpsimd.dma_start(b2_sb, b2)
            w1_sb = [
                consts.tile([D_IN, NCHUNK, 128], BF16, name=f"w1e{e}", tag=f"w1e{e}")
                for e in range(E)
            ]
            w2_sb = [
                consts.tile([128, NCHUNK, D_OUT], BF16, name=f"w2e{e}", tag=f"w2e{e}")
                for e in range(E)
            ]
            for e in range(4):
                nc.gpsimd.dma_start(
                    w1_sb[e], w1l[:, NCHUNK * e : NCHUNK * (e + 1), :]
                )
            # ACT queue: a few early consts (its first relu isn't until
            # ~3us), then the b1 launder so per-tile relus depend on the
            # Activation sem instead of a DMA sem.
            for e in (0, 1):
                nc.scalar.dma_start(
                    w2_sb[e], w2l[:, NCHUNK * e : NCHUNK * (e + 1), :]
                )
            for e in (4, 5):
                nc.scalar.dma_start(
                    w1_sb[e], w1l[:, NCHUNK * e : NCHUNK * (e + 1), :]
                )
            b1_sb = consts.tile([128, E * NCHUNK], F32, tag="b1_act")
            nc.scalar.copy(b1_sb, b1_dma)
            # Remaining consts ride the SP queue between tile input DMAs.

            # ---- per-tile state ----
            xa = [None] * n_tiles  # xt halves
            xb = [None] * n_tiles
            wt_sb = [None] * n_tiles
            wbc = [[None] * E for _ in range(n_tiles)]
            hs_t = [[None] * E for _ in range(n_tiles)]
            po = [None] * n_tiles

            def dma_inputs(t):
                b0 = t * nb
                xa[t] = xt_p.tile([D_IN, 512], BF16, name="xa", tag="xa")
                nc.sync.dma_start(xa[t], xt[:, b0 : b0 + 512])
                xb[t] = xt_p.tile([D_IN, 512], BF16, name="xb", tag="xb")
                nc.sync.dma_start(xb[t], xt[:, b0 + 512 : b0 + nb])
                wt_sb[t] = wt_p.tile([E, nb], BF16, name="wt_sb")
                nc.sync.dma_start(wt_sb[t], wt[:, b0 : b0 + nb])

            def dma_wbc(t, e):
                b0 = t * nb
                wbc[t][e] = wbc_p.tile([128, nb], BF16, name="wbc")
                nc.sync.dma_start(
                    wbc[t][e],
                    wt[e : e + 1, b0 : b0 + nb].partition_broadcast(128),
                )

            def emit_chunk(t, e, m, h):
                """L1 matmuls + relu for chunk (t, e, m) into h[:, m, :]."""
                c = NCHUNK * e + m
                z = z_ps.tile([128, nb], F32, tag="z")
                for j, xh in enumerate((xa[t], xb[t])):
                    sl = slice(j * 512, (j + 1) * 512)
                    nc.tensor.matmul(
                        z[:, sl], lhsT=w1_sb[e][:, m, :], rhs=xh,
                        start=True, stop=True,
                    )
                if c in RELU_JSPLIT:
                    nc.scalar.activation(
                        h[:, m, :512], z[:, :512], RELU,
                        bias=b1_sb[:, c : c + 1], scale=1.0,
                    )
                    nc.vector.tensor_scalar(
                        h[:, m, 512:], z[:, 512:],
                        b1_sb[:, c : c + 1], 0.0,
                        mybir.AluOpType.add, mybir.AluOpType.max,
                    )
                elif c in RELU_DVE:
                    nc.vector.tensor_scalar(
                        h[:, m, :], z,
                        b1_sb[:, c : c + 1], 0.0,
                        mybir.AluOpType.add, mybir.AluOpType.max,
                    )
                else:
                    nc.scalar.activation(
                        h[:, m, :], z, RELU,
                        bias=b1_sb[:, c : c + 1], scale=1.0,
                    )

            def emit_mul(t, e, h, hs):
                if e in MUL_POOL:
                    # four [128,512] pieces on the Pool engine (keeps each
                    # piece's latency ~1us so the lag-2 budget holds)
                    for m in range(NCHUNK):
                        for j in range(nsub):
                            sl = slice(j * 512, (j + 1) * 512)
                            nc.gpsimd.tensor_mul(
                                hs[:, m, sl], h[:, m, sl], wbc[t][e][:, sl]
                            )
                else:
                    nc.vector.tensor_mul(hs, h, _rep2(wbc[t][e], NCHUNK))

            def emit_L1(t, e, defer_mul=False):
                """L1 + relus + multiply for (t, e); optionally defer the mul."""
                h = h_p.tile([128, NCHUNK, nb], BF16)
                hs = hs_p.tile([128, NCHUNK, nb], BF16)
                hs_t[t][e] = hs
                for m in range(NCHUNK):
                    emit_chunk(t, e, m, h)
                mul = lambda: emit_mul(t, e, h, hs)
                if defer_mul:
                    return mul
                mul()
                return None

            def emit_b2init(t):
                po[t] = o_ps.tile([D_OUT, nb], F32, name="po")
                for j in range(nsub):
                    sl = slice(j * 512, (j + 1) * 512)
                    nc.tensor.matmul(
                        po[t][:, sl], lhsT=b2_sb, rhs=wt_sb[t][:, sl],
                        start=True, stop=False,
                    )

            def emit_L2(t, e):
                hs = hs_t[t][e]
                for k in range(NCHUNK):
                    for j in range(nsub):
                        sl = slice(j * 512, (j + 1) * 512)
                        nc.tensor.matmul(
                            po[t][:, sl], lhsT=w2_sb[e][:, k, :],
                            rhs=hs[:, k, sl],
                            start=False,
                            stop=(e == E - 1 and k == NCHUNK - 1),
                        )
                hs_t[t][e] = None

            def emit_copyA(t):
                b0 = t * nb
                ota = ot_p.tile([D_OUT, 512], BF16, tag="ota")
                nc.scalar.copy(ota, po[t][:, :512])
                nc.sync.dma_start(outT[:, b0 : b0 + 512], ota)

            def emit_copyB(t):
                b0 = t * nb
                otb = ot_p.tile([D_OUT, 512], BF16, tag="otb")
                nc.vector.tensor_scalar_add(otb, po[t][:, 512:], 0.0)
                nc.sync.dma_start(outT[:, b0 + 512 : b0 + nb], otb)

            # ---- software-pipelined emission (L2 lags L1 by 3 slots) ----
            # PE slot map (tile t, slot e):
            #   (t,0): L1(t,0); L2(t-1,5)
            #   (t,1): L2(t-1,6); L1(t,1)
            #   (t,2): L2(t-1,7); L1(t,2)    <- po(t-1) stops early here,
            #          then drain(t-1): ACT queue [copyA, relu c4, ...],
            #          DVE queue [copyB, relu c5, ...] - the copies never
            #          head-block ready relus, and po is free ~1.5us before
            #          b2init(t) needs it in slot 3.
            #   (t,3): L1(t,3); b2init(t); L2(t,0)
            #   (t,s>=4): L1(t,s); L2(t,s-3)
            # The 3-slot lag gives every relu->multiply chain 3.4-5us of
            # slack, enough even for the Pool-engine multiplies.
            dma_inputs(0)
            for e in (2, 3):
                nc.sync.dma_start(
                    w2_sb[e], w2l[:, NCHUNK * e : NCHUNK * (e + 1), :]
                )
            for e in (6, 7):
                nc.sync.dma_start(
                    w1_sb[e], w1l[:, NCHUNK * e : NCHUNK * (e + 1), :]
                )
            for e in range(3):
                dma_wbc(0, e)
            for t in range(n_tiles):
                if t + 1 < n_tiles:
                    dma_inputs(t + 1)
                if t == 0:
                    for e in (4, 5, 6, 7):
                        nc.sync.dma_start(
                            w2_sb[e], w2l[:, NCHUNK * e : NCHUNK * (e + 1), :]
                        )
                for e in range(E):
                    if e + 3 < E:
                        dma_wbc(t, e + 3)
                    elif t + 1 < n_tiles:
                        dma_wbc(t + 1, e + 3 - E)
                    if e == 0:
                        emit_L1(t, 0)
                        if t > 0:
                            emit_L2(t - 1, 5)
                    elif e == 1:
                        if t > 0:
                            emit_L2(t - 1, 6)
                        emit_L1(t, 1)
                    elif e == 2:
                        if t > 0:
                            emit_L2(t - 1, 7)
                            emit_copyA(t - 1)
                            emit_copyB(t - 1)
                        emit_L1(t, 2)
                    elif e == 3:
                        emit_L1(t, 3)
                        emit_b2init(t)
                        emit_L2(t, 0)
                    else:
                        emit_L1(t, e)
                        emit_L2(t, e - 3)
            t = n_tiles - 1
            emit_L2(t, 5)
            emit_L2(t, 6)
            emit_L2(t, 7)
            emit_copyA(t)
            emit_copyB(t)
    dedup_ldw(nc)
    return legalize_waits(nc) if legalize else nc


def prep_consts(W1, b1, W2, b2):
    bf = ml_dtypes.bfloat16
    # w1l[i, (e, m), h'] = W1[e, i, m*128 + h']
    w1l = np.ascontiguousarray(
        W1.transpose(1, 0, 2).reshape(D_IN, E, NCHUNK, 128).reshape(D_IN, E * NCHUNK, 128)
    ).astype(bf)
    # b1l[p, (e, m)] = b1[e, m*128 + p]
    b1l = np.ascontiguousarray(
        b1.reshape(E, NCHUNK, 128).transpose(2, 0, 1).reshape(128, E * NCHUNK)
    ).astype(np.float32)
    # w2l[h', (e, k), o] = W2[e, k*128 + h', o]
    w2l = np.ascontiguousarray(
        W2.reshape(E, NCHUNK, 128, D_OUT).transpose(2, 0, 1, 3).reshape(128, E * NCHUNK, D_OUT)
    ).astype(bf)
    return {"w1l": w1l, "b1l": b1l, "w2l": w2l, "b2": b2.astype(bf)}


def prep_core(x_c, w_c, consts, b_shard):
    bf = ml_dtypes.bfloat16
    xt = np.ascontiguousarray(x_c.T).astype(bf)
    wt = np.ascontiguousarray(w_c.T).astype(bf)
    return {"xt": xt, "wt": wt, **consts}


def _ntff_hook():
    """NTFF profiling hook via the axon PJRT .so (the antenv.axon_hooks
    glue module is absent in this image, so wire it up directly)."""
    from trn_agent_boot.trn_boot import _ntff_profile_via_ctypes

    return _ntff_profile_via_ctypes("/opt/axon/libaxon_pjrt.so")


def run_traced(nc, in_maps, n_cores, out_dir):
    import concourse.bass2jax as bass2jax

    hook = _ntff_hook()
    with hook(out_dir, list(range(n_cores))):
        results = bass2jax.run_bass_via_pjrt(nc, in_maps, n_cores=n_cores)
    return results


def run(inputs, trace=False, b_shard=B_SHARD, nb=NB):
    x = np.asarray(inputs["x"], dtype=np.float32)
    w = np.asarray(inputs["weights"], dtype=np.float32)
    consts = prep_consts(
        np.asarray(inputs["W1"], dtype=np.float32),
        np.asarray(inputs["b1"], dtype=np.float32),
        np.asarray(inputs["W2"], dtype=np.float32),
        np.asarray(inputs["b2"], dtype=np.float32),
    )
    n_cores = x.shape[0] // b_shard
    key = (b_shard, nb)
    if key not in _nc_cache:
        _nc_cache[key] = build_nc(b_shard, nb)
    nc = _nc_cache[key]
    in_maps = [
        prep_core(
            x[c * b_shard : (c + 1) * b_shard],
            w[c * b_shard : (c + 1) * b_shard],
            consts,
            b_shard,
        )
        for c in range(n_cores)
    ]
    if trace:
        import tempfile

        out_dir = tempfile.mkdtemp(prefix="moe_ntff_")
        results = run_traced(nc, in_maps, n_cores, out_dir)

        class _Res:
            pass

        res = _Res()
        res.results = results
        res.exec_time_ns = None
        res.trace_dir = out_dir
    else:
        res = run_bass_kernel_spmd(
            nc, in_maps, core_ids=list(range(n_cores)), trace=False
        )
        res.trace_dir = None
    out = np.concatenate([np.ascontiguousarray(r["outT"].T) for r in res.results], axis=0)
    return out.astype(np.float32), res


def kernel(**inputs) -> np.ndarray:
    out, _ = run(inputs)
    return out


# revision 26
# speedup vs baseline: 1.3885x; 1.3885x over previous
# MoE layer (all-experts dense MLP + weighted combine) on 8 TRN2 NeuronCores.
#
# Reference, for every token b (B=65536 total):
#   h_e   = relu(x @ W1[e] + b1[e])          e = 0..7
#   y_e   = h_e @ W2[e] + b2[e]
#   out_b = sum_e weights[b, e] * y_e
#
# Strategy (data-parallel over B, expert params replicated):
#   - Shard B across the 8 cores (8192 tokens each).
#   - Hidden dim stays on partitions ("hdim-major"):
#       L1:  z_e^T[h, b]  = W1_e^T @ x^T          (W1 chunks stationary)
#       h_e^T             = relu(z_e^T + b1_e)    (ACT/DVE, per-partition bias)
#       hs_e^T            = h_e^T * w_bcast_e     (DVE)
#       out^T[o, b]       = sum_{e,k} W2_chunk^T @ hs_chunk + b2^T @ w^T
#     accumulated in a single PSUM bank - the expert combine is free.
#   - w_bcast_e comes from a DMA with a step-0 partition AP reading the
#     host-transposed weights row straight from DRAM.
#
# v2 scheduling (driven by the neuron-profile trace of v1):
#   - Const weights are DMA'd in per-expert chunks spread over the Pool /
#     DVE / SP queues in first-use order, so the first L1 matmul fires at
#     ~2us instead of ~23us (v1 serialized 1MB of consts on one queue).
#   - The PE stream is software-pipelined ACROSS tiles: L1(t,e) leads
#     L2(t,e-1), and tile t+1's first L1 runs before tile t's last L2, so
#     the relu->mul chain of the last expert no longer stalls the PE at
#     every tile boundary (~1.3us/tile in v1).
#   - The PSUM out-accumulator drain is split in halves on ACT and DVE so
#     the single po buffer recycles ~0.6us faster.
#   - x^T tiles are split into 512-token halves so the first matmul only
#     waits for 128KB, and the out DMA starts after the first half-copy.
import numpy as np
import ml_dtypes

import concourse.bass as bass
import concourse.mybir as mybir
import concourse.tile as tile
import concourse.bass_utils as _bu
from concourse.bass_utils import run_bass_kernel_spmd


E, D_IN, D_HID, D_OUT, B = 8, 128, 256, 128, 65536
N_CORES = 8
B_SHARD = B // N_CORES  # 8192
NB = 1024               # tokens per tile
NCHUNK = D_HID // 128   # 2 hidden-dim chunks per expert

BF16 = mybir.dt.bfloat16
F32 = mybir.dt.float32
RELU = mybir.ActivationFunctionType.Relu

# engine split knobs (tuned against the profile)
# relu chunks (c = 2*e + m) assigned fully to DVE; the rest go to ACT.
RELU_DVE = {5, 9, 13}
# relu chunks split in 512-token halves: j0 on ACT, j1 on DVE.
RELU_JSPLIT = {15}
# experts whose h*w multiply runs on GpSimd pieces. Measured: Pool TT is
# ~1.4us per [128,512] piece and the pieces serialize, blowing the L2
# deadline -> keep empty; Pool instead builds the wbc broadcasts.
MUL_POOL = set()
# build wbc with GpSimd partition_broadcast (attn ucode library) instead
# of a broadcast-DMA. Tried and CRASHED the device (NRT_EXEC_UNIT_
# UNRECOVERABLE) - the ucode library swap is not supported in this flow.
# Keep False.
WBC_POOL = False

_nc_cache = {}


def dedup_ldw(nc):
    """Drop redundant PE weight loads.

    Tile emits an InstLdweights before every InstMatmult; consecutive
    matmuls over the two 512-token halves of a tile reuse the same
    stationary weights, so the second load is a hardware no-op (weights
    persist in the PE array until the next load). Deleting it saves PE
    queue time; its semaphore waits/updates are carried onto the next PE
    instruction (legalize_waits splits any overflow afterwards).
    """
    for f in nc.m.functions:
        for b in f.blocks:
            il = b.instructions
            out = []
            last_key = None
            carry_w, carry_u = [], []
            for inst in il:
                if inst.engine != mybir.EngineType.PE:
                    out.append(inst)
                    continue
                if isinstance(inst, mybir.InstLdweights):
                    key = str(inst.ins[0])
                    if key == last_key:
                        si = inst.sync_info
                        if si is not None:
                            carry_w.extend(list(si.on_wait))
                            carry_u.extend(list(si.on_update))
                        continue
                    last_key = key
                elif not isinstance(
                    inst, (mybir.InstMatmult, mybir.InstEventSemaphore)
                ):
                    last_key = None
                if carry_w or carry_u:
                    si = inst.sync_info
                    w = (list(si.on_wait) if si else []) + carry_w
                    u = (list(si.on_update) if si else []) + carry_u
                    inst.sync_info = mybir.SyncInfo(on_wait=w, on_update=u)
                    carry_w, carry_u = [], []
                out.append(inst)
            il[:] = out
    return nc


def legalize_waits(nc):
    """Split multi-wait instructions into standalone EventSemaphore waits.

    The walrus build in this container enforces the hardware sync-slot
    budget strictly: a normal instruction holds at most 1 sem wait (+1
    update); an EventSemaphore instruction holds 2. Tile's scheduler
    attaches up to 3 waits per instruction (and ~11 on the kernel-tail
    drain), which codegen rejects with "Too many sync wait commands".
    Hoisting the excess waits into standalone EventSemaphore instructions
    immediately before the op (same engine queue, so they gate execution
    identically) makes the program legal without changing semantics.
    """
    for f in nc.m.functions:
        for b in f.blocks:
            il = b.instructions
            out = []
            changed = False
            for inst in il:
                si = inst.sync_info
                if si is not None:
                    waits = list(si.on_wait)
                    upds = list(si.on_update)
                    assert len(upds) <= 1, f"{inst.name}: {len(upds)} updates"
                    cap = 2 if isinstance(inst, mybir.InstEventSemaphore) else 1
                    if len(waits) > cap:
                        extra, keep = waits[:-cap], waits[-cap:]
                        k = 0
                        while extra:
                            chunk, extra = extra[:2], extra[2:]
                            ev = mybir.InstEventSemaphore(
                                name=f"{inst.name}-lw{k}", ins=[], outs=[]
                            )
                            ev.engine = inst.engine
                            ev.sync_info = mybir.SyncInfo(
                                on_wait=chunk, on_update=[]
                            )
                            out.append(ev)
                            k += 1
                        inst.sync_info = mybir.SyncInfo(
                            on_wait=keep, on_update=upds
                        )
                        changed = True
                out.append(inst)
            if changed:
                il[:] = out
    return nc


def _rep2(ap_2d, n):
    """View a [128, F] AP as [128, n, F] with a step-0 middle dim."""
    return bass.AP(
        tensor=ap_2d.tensor,
        offset=ap_2d.offset,
        ap=[ap_2d.ap[0], [0, n], ap_2d.ap[1]],
    )


def build_nc(b_shard=B_SHARD, nb=NB, legalize=True):
    assert b_shard % nb == 0
    n_tiles = b_shard // nb
    nsub = nb // 512
    assert nsub == 2, "schedule assumes 1024-token tiles"
    nc = bass.Bass(trn_type="TRN2")

    xt = nc.dram_tensor("xt", [D_IN, b_shard], BF16, kind="ExternalInput").ap()
    wt = nc.dram_tensor("wt", [E, b_shard], BF16, kind="ExternalInput").ap()
    # W1 laid out [i, (e, m), h']: chunk (e, m) is lhsT for z_e rows m*128..
    w1l = nc.dram_tensor("w1l", [D_IN, E * NCHUNK, 128], BF16, kind="ExternalInput").ap()
    # b1 laid out [p, (e, m)] = b1[e, m*128 + p]
    b1l = nc.dram_tensor("b1l", [128, E * NCHUNK], F32, kind="ExternalInput").ap()
    # W2 laid out [h', (e, k), o]: chunk (e, k) is lhsT contracting h rows k*128..
    w2l = nc.dram_tensor("w2l", [128, E * NCHUNK, D_OUT], BF16, kind="ExternalInput").ap()
    b2 = nc.dram_tensor("b2", [E, D_OUT], BF16, kind="ExternalInput").ap()
    # out is produced in bf16 (the host upcasts); this halves the output DMA
    # bytes on a DMA fabric that is within ~10% of saturation, and costs
    # ~0.2% relative error against a 2% gate.
    outT = nc.dram_tensor("outT", [D_OUT, b_shard], BF16, kind="ExternalOutput").ap()

    with tile.TileContext(nc) as tc:
        with (
            tc.tile_pool(name="consts", bufs=1) as consts,
            tc.tile_pool(name="xt_p", bufs=6) as xt_p,
            tc.tile_pool(name="wt_p", bufs=3) as wt_p,
            tc.tile_pool(name="wbc_p", bufs=6) as wbc_p,
            tc.tile_pool(name="wtr_p", bufs=6) as wtr_p,
            tc.tile_pool(name="h_p", bufs=6) as h_p,
            tc.tile_pool(name="hs_p", bufs=6) as hs_p,
            tc.tile_pool(name="ot_p", bufs=6) as ot_p,
            tc.tile_pool(name="z_ps", bufs=3, space="PSUM") as z_ps,
            tc.tile_pool(name="o_ps", bufs=1, space="PSUM") as o_ps,
        ):
            # ---- consts, chunked + spread over queues in first-use order ----
            if WBC_POOL:
                from concourse import library_config
                nc.gpsimd.load_library(library_config.attn)
            # Pool queue: b1, b2, W1 for e0-e3, then the wbc stream (below).
            b1_dma = consts.tile([128, E * NCHUNK], F32, tag="b1_dma")
            nc.gpsimd.dma_start(b1_dma, b1l)
            b2_sb = consts.tile([E, D_OUT], BF16, tag="b2")
            nc.gpsimd.dma_start(b2_sb, b2)
            w1_sb = [
                consts.tile([D_IN, NCHUNK, 128], BF16, name=f"w1e{e}", tag=f"w1e{e}")
                for e in range(E)
            ]
            w2_sb = [
                consts.tile([128, NCHUNK, D_OUT], BF16, name=f"w2e{e}", tag=f"w2e{e}")
                for e in range(E)
            ]
            for e in range(4):
                nc.gpsimd.dma_start(
                    w1_sb[e], w1l[:, NCHUNK * e : NCHUNK * (e + 1), :]
                )
            # ACT queue: a few early consts (its first relu isn't until
            # ~3us), then the b1 launder so per-tile relus depend on the
            # Activation sem instead of a DMA sem.
            for e in (0, 1):
                nc.scalar.dma_start(
                    w2_sb[e], w2l[:, NCHUNK * e : NCHUNK * (e + 1), :]
                )
            for e in (4, 5):
                nc.scalar.dma_start(
                    w1_sb[e], w1l[:, NCHUNK * e : NCHUNK * (e + 1), :]
                )
            b1_sb = consts.tile([128, E * NCHUNK], F32, tag="b1_act")
            nc.scalar.copy(b1_sb, b1_dma)
            # Remaining consts ride the SP queue between tile input DMAs.

            # ---- per-tile state ----
            xa = [None] * n_tiles  # xt halves
            xb = [None] * n_tiles
            wt_sb = [None] * n_tiles
            wbc = [[None] * E for _ in range(n_tiles)]
            hs_t = [[None] * E for _ in range(n_tiles)]
            po = [None] * n_tiles

            def dma_inputs(t):
                b0 = t * nb
                xa[t] = xt_p.tile([D_IN, 512], BF16, name="xa", tag="xa")
                nc.sync.dma_start(xa[t], xt[:, b0 : b0 + 512])
                xb[t] = xt_p.tile([D_IN, 512], BF16, name="xb", tag="xb")
                nc.sync.dma_start(xb[t], xt[:, b0 + 512 : b0 + nb])
                wt_sb[t] = wt_p.tile([E, nb], BF16, name="wt_sb")
                nc.sync.dma_start(wt_sb[t], wt[:, b0 : b0 + nb])

            def dma_wbc(t, e):
                b0 = t * nb
                wbc[t][e] = wbc_p.tile([128, nb], BF16, name="wbc")
                if WBC_POOL:
                    wtrow = wtr_p.tile([1, nb], BF16, name="wtrow")
                    nc.sync.dma_start(wtrow, wt[e : e + 1, b0 : b0 + nb])
                    nc.gpsimd.partition_broadcast(wbc[t][e], wtrow, 128)
                else:
                    nc.gpsimd.dma_start(
                        wbc[t][e],
                        wt[e : e + 1, b0 : b0 + nb].partition_broadcast(128),
                    )

            def emit_chunk(t, e, m, h):
                """L1 matmuls + relu for chunk (t, e, m) into h[:, m, :]."""
                c = NCHUNK * e + m
                z = z_ps.tile([128, nb], F32, tag="z")
                for j, xh in enumerate((xa[t], xb[t])):
                    sl = slice(j * 512, (j + 1) * 512)
                    nc.tensor.matmul(
                        z[:, sl], lhsT=w1_sb[e][:, m, :], rhs=xh,
                        start=True, stop=True,
                    )
                if c in RELU_JSPLIT:
                    nc.scalar.activation(
                        h[:, m, :512], z[:, :512], RELU,
                        bias=b1_sb[:, c : c + 1], scale=1.0,
                    )
                    nc.vector.tensor_scalar(
                        h[:, m, 512:], z[:, 512:],
                        b1_sb[:, c : c + 1], 0.0,
                        mybir.AluOpType.add, mybir.AluOpType.max,
                    )
                elif c in RELU_DVE:
                    nc.vector.tensor_scalar(
                        h[:, m, :], z,
                        b1_sb[:, c : c + 1], 0.0,
                        mybir.AluOpType.add, mybir.AluOpType.max,
                    )
                else:
                    nc.scalar.activation(
                        h[:, m, :], z, RELU,
                        bias=b1_sb[:, c : c + 1], scale=1.0,
                    )

            def emit_mul(t, e, h, hs):
                if e in MUL_POOL:
                    # four [128,512] pieces on the Pool engine (keeps each
                    # piece's latency ~1us so the lag-2 budget holds)
                    for m in range(NCHUNK):
                        for j in range(nsub):
                            sl = slice(j * 512, (j + 1) * 512)
                            nc.gpsimd.tensor_mul(
                                hs[:, m, sl], h[:, m, sl], wbc[t][e][:, sl]
                            )
                else:
                    nc.vector.tensor_mul(hs, h, _rep2(wbc[t][e], NCHUNK))

            def emit_L1(t, e, defer_mul=False):
                """L1 + relus + multiply for (t, e); optionally defer the mul."""
                h = h_p.tile([128, NCHUNK, nb], BF16)
                hs = hs_p.tile([128, NCHUNK, nb], BF16)
                hs_t[t][e] = hs
                for m in range(NCHUNK):
                    emit_chunk(t, e, m, h)
                mul = lambda: emit_mul(t, e, h, hs)
                if defer_mul:
                    return mul
                mul()
                return None

            def emit_b2init(t):
                po[t] = o_ps.tile([D_OUT, nb], F32, name="po")
                for j in range(nsub):
                    sl = slice(j * 512, (j + 1) * 512)
                    nc.tensor.matmul(
                        po[t][:, sl], lhsT=b2_sb, rhs=wt_sb[t][:, sl],
                        start=True, stop=False,
                    )

            def emit_L2(t, e):
                hs = hs_t[t][e]
                for k in range(NCHUNK):
                    for j in range(nsub):
                        sl = slice(j * 512, (j + 1) * 512)
                        nc.tensor.matmul(
                            po[t][:, sl], lhsT=w2_sb[e][:, k, :],
                            rhs=hs[:, k, sl],
                            start=False,
                            stop=(e == E - 1 and k == NCHUNK - 1),
                        )
                hs_t[t][e] = None

            def emit_copyA(t):
                b0 = t * nb
                ota = ot_p.tile([D_OUT, 512], BF16, tag="ota")
                nc.scalar.copy(ota, po[t][:, :512])
                nc.sync.dma_start(outT[:, b0 : b0 + 512], ota)

            def emit_copyB(t):
                b0 = t * nb
                otb = ot_p.tile([D_OUT, 512], BF16, tag="otb")
                nc.vector.tensor_scalar_add(otb, po[t][:, 512:], 0.0)
                nc.sync.dma_start(outT[:, b0 + 512 : b0 + nb], otb)

            # ---- software-pipelined emission (L2 lags L1 by 3 slots) ----
            # PE slot map (tile t, slot e):
            #   (t,0): L1(t,0); L2(t-1,5)
            #   (t,1): L2(t-1,6); L1(t,1)
            #   (t,2): L2(t-1,7); L1(t,2)    <- po(t-1) stops early here,
            #          then drain(t-1): ACT queue [copyA, relu c4, ...],
            #          DVE queue [copyB, relu c5, ...] - the copies never
            #          head-block ready relus, and po is free ~1.5us before
            #          b2init(t) needs it in slot 3.
            #   (t,3): L1(t,3); b2init(t); L2(t,0)
            #   (t,s>=4): L1(t,s); L2(t,s-3)
            # The 3-slot lag gives every relu->multiply chain 3.4-5us of
            # slack, enough even for the Pool-engine multiplies.
            dma_inputs(0)
            for e in (2, 3):
                nc.sync.dma_start(
                    w2_sb[e], w2l[:, NCHUNK * e : NCHUNK * (e + 1), :]
                )
            for e in (6, 7):
                nc.sync.dma_start(
                    w1_sb[e], w1l[:, NCHUNK * e : NCHUNK * (e + 1), :]
                )
            for e in range(3):
                dma_wbc(0, e)
            for t in range(n_tiles):
                if t + 1 < n_tiles:
                    dma_inputs(t + 1)
                if t == 0:
                    for e in (4, 5, 6, 7):
                        nc.sync.dma_start(
                            w2_sb[e], w2l[:, NCHUNK * e : NCHUNK * (e + 1), :]
                        )
                for e in range(E):
                    if e + 3 < E:
                        dma_wbc(t, e + 3)
                    elif t + 1 < n_tiles:
                        dma_wbc(t + 1, e + 3 - E)
                    if e == 0:
                        emit_L1(t, 0)
                        if t > 0:
                            emit_L2(t - 1, 5)
                    elif e == 1:
                        if t > 0:
                            emit_L2(t - 1, 6)
                        emit_L1(t, 1)
                    elif e == 2:
                        if t > 0:
                            emit_L2(t - 1, 7)
                            emit_copyA(t - 1)
                            emit_copyB(t - 1)
                        emit_L1(t, 2)
                    elif e == 3:
                        emit_L1(t, 3)
                        emit_b2init(t)
                        emit_L2(t, 0)
                    else:
                        emit_L1(t, e)
                        emit_L2(t, e - 3)
            t = n_tiles - 1
            emit_L2(t, 5)
            emit_L2(t, 6)
            emit_L2(t, 7)
            emit_copyA(t)
            emit_copyB(t)
    dedup_ldw(nc)
    return legalize_waits(nc) if legalize else nc


def prep_consts(W1, b1, W2, b2):
    bf = ml_dtypes.bfloat16
    # w1l[i, (e, m), h'] = W1[e, i, m*128 + h']
    w1l = np.ascontiguousarray(
        W1.transpose(1, 0, 2).reshape(D_IN, E, NCHUNK, 128).reshape(D_IN, E * NCHUNK, 128)
    ).astype(bf)
    # b1l[p, (e, m)] = b1[e, m*128 + p]
    b1l = np.ascontiguousarray(
        b1.reshape(E, NCHUNK, 128).transpose(2, 0, 1).reshape(128, E * NCHUNK)
    ).astype(np.float32)
    # w2l[h', (e, k), o] = W2[e, k*128 + h', o]
    w2l = np.ascontiguousarray(
        W2.reshape(E, NCHUNK, 128, D_OUT).transpose(2, 0, 1, 3).reshape(128, E * NCHUNK, D_OUT)
    ).astype(bf)
    return {"w1l": w1l, "b1l": b1l, "w2l": w2l, "b2": b2.astype(bf)}


def prep_core(x_c, w_c, consts, b_shard):
    bf = ml_dtypes.bfloat16
    xt = np.ascontiguousarray(x_c.T).astype(bf)
    wt = np.ascontiguousarray(w_c.T).astype(bf)
    return {"xt": xt, "wt": wt, **consts}


def _ntff_hook():
    """NTFF profiling hook via the axon PJRT .so (the antenv.axon_hooks
    glue module is absent in this image, so wire it up directly)."""
    from trn_agent_boot.trn_boot import _ntff_profile_via_ctypes

    return _ntff_profile_via_ctypes("/opt/axon/libaxon_pjrt.so")


def run_traced(nc, in_maps, n_cores, out_dir):
    import concourse.bass2jax as bass2jax

    hook = _ntff_hook()
    with hook(out_dir, list(range(n_cores))):
        results = bass2jax.run_bass_via_pjrt(nc, in_maps, n_cores=n_cores)
    return results


def run(inputs, trace=False, b_shard=B_SHARD, nb=NB):
    x = np.asarray(inputs["x"], dtype=np.float32)
    w = np.asarray(inputs["weights"], dtype=np.float32)
    consts = prep_consts(
        np.asarray(inputs["W1"], dtype=np.float32),
        np.asarray(inputs["b1"], dtype=np.float32),
        np.asarray(inputs["W2"], dtype=np.float32),
        np.asarray(inputs["b2"], dtype=np.float32),
    )
    n_cores = x.shape[0] // b_shard
    key = (b_shard, nb)
    if key not in _nc_cache:
        _nc_cache[key] = build_nc(b_shard, nb)
    nc = _nc_cache[key]
    in_maps = [
        prep_core(
            x[c * b_shard : (c + 1) * b_shard],
            w[c * b_shard : (c + 1) * b_shard],
            consts,
            b_shard,
        )
        for c in range(n_cores)
    ]
    if trace:
        import tempfile

        out_dir = tempfile.mkdtemp(prefix="moe_ntff_")
        results = run_traced(nc, in_maps, n_cores, out_dir)

        class _Res:
            pass

        res = _Res()
        res.results = results
        res.exec_time_ns = None
        res.trace_dir = out_dir
    else:
        res = run_bass_kernel_spmd(
            nc, in_maps, core_ids=list(range(n_cores)), trace=False
        )
        res.trace_dir = None
    out = np.concatenate([np.ascontiguousarray(r["outT"].T) for r in res.results], axis=0)
    return out.astype(np.float32), res


def kernel(**inputs) -> np.ndarray:
    out, _ = run(inputs)
    return out
